# revision 3
# baseline (speedup 1.0000x reference)
"""Multi-head causal attention (RoPE) for Trainium2, sharded over 8 NeuronCores.

Sharding: core c = 4*b + g handles batch b (of 2) and head-group g (4 of 16 heads).
Each core computes the qkv projection for its heads, RoPE, causal attention, and
a partial output projection (row-parallel over its heads' dims). The host sums
the 4 partials per batch and adds proj_b (plus the folded V-bias term).

v2 changes vs baseline:
  - fp16 operands everywhere on-chip (PSUM accumulation stays fp32): halves DMA
    and SBUF traffic, 2x DVE throughput on RoPE elementwise ops, and allows
    128-wide matmuls at 1 cycle/row (fp32r would be 4x for moving<256).
  - fine-grained causal diagonal: crossing k-tiles only compute the valid q
    range (start=True resets the whole PSUM bank, zeroing the rest), and the
    triangular 128x128 sub-block mask is injected by one extra 128-moving
    matmul: psS += triA.T @ (-60000*I) adds -60000 exactly where q < k, so
    exp() flushes those to zero. This removes the gpsimd affine_select from
    the exp->PV critical path entirely and keeps masking PE-local.
  - V bias folded into the host-side combine (softmax weights sum to 1, so the
    bias contributes bv @ proj_w.T, a constant row); removes the normalize-time
    bias adds.
  - x is loaded fully resident as 8 [128, 2048] fp16 chunks (big DMAs),
    interleaved with the qkv weight chunks so the first matmul starts early.
  - proj PSUM shares a pool with the attention output accumulators (disjoint
    lifetimes), proj output is staged as one [128, 1024] tile per 128-row
    block -> one output DMA per block.
"""
import numpy as np

B, S, D = 2, 2048, 1024
HEADS, HD = 16, 64
HALF = HD // 2
NCORES = 8
GROUPS = 4          # tensor-parallel head groups per batch
HPG = HEADS // GROUPS
O_QK = 2 * HPG * HD   # 512 rows: [q h0..h3 | k h0..h3]
O_V = HPG * HD        # 256
SCALE = 1.0 / np.sqrt(HD)

NST = S // 512        # 4 seq tiles of 512
NDC = D // 128        # 8 contraction chunks
NKT = S // 128        # 16 k tiles

_NC_CACHE = None

# schedule knobs
PSS_BUFS = 2
E_BUFS = 7
RESERVE = 2   # fillers kept back to cover the normalize window
P1_BUFS = 3
NRM_BUFS = 2
SPLIT_EXP_QT3 = False
SPLIT_EXP_CROSS = False


def _build_nc():
    import concourse.bacc as bacc
    import concourse.mybir as mybir
    import concourse.tile as tile

    fp32 = mybir.dt.float32
    fp16 = mybir.dt.float16
    Exp = mybir.ActivationFunctionType.Exp

    nc = bacc.Bacc(trn_type="TRN2", target_bir_lowering=False, debug=False)

    xT = nc.dram_tensor("xT", [D, S], fp16, kind="ExternalInput").ap()
    wqkT = nc.dram_tensor("wqkT", [D, O_QK], fp16, kind="ExternalInput").ap()
    wvT = nc.dram_tensor("wvT", [D, O_V], fp16, kind="ExternalInput").ap()
    bqk = nc.dram_tensor("bqk", [128, 4], fp32, kind="ExternalInput").ap()
    cosF = nc.dram_tensor("cosF", [128, S], fp16, kind="ExternalInput").ap()
    sinF = nc.dram_tensor("sinF", [128, S], fp16, kind="ExternalInput").ap()
    pwT = nc.dram_tensor("pwT", [O_V, D], fp16, kind="ExternalInput").ap()
    triA = nc.dram_tensor("triA", [128, 128], fp16, kind="ExternalInput").ap()
    negI = nc.dram_tensor("negI", [128, 128], fp16, kind="ExternalInput").ap()
    outP = nc.dram_tensor("out_partial", [S, D], fp16, kind="ExternalOutput").ap()

    with tile.TileContext(nc) as tc:
        with tc.tile_pool(name="persist", bufs=1) as persist, \
             tc.tile_pool(name="p1sb", bufs=P1_BUFS) as p1sb, \
             tc.tile_pool(name="p2sb", bufs=E_BUFS) as p2sb, \
             tc.tile_pool(name="nrm", bufs=NRM_BUFS) as nrm, \
             tc.tile_pool(name="p3sb", bufs=4) as p3sb, \
             tc.tile_pool(name="ps1", bufs=2, space="PSUM") as ps1_pool, \
             tc.tile_pool(name="psS", bufs=PSS_BUFS, space="PSUM") as psS_pool, \
             tc.tile_pool(name="psX", bufs=2, space="PSUM") as psX_pool:

            # --- persistent tiles -------------------------------------------
            wqk_t = [persist.tile([128, O_QK], fp16, name=f"wqk{dc}", tag=f"wqk{dc}")
                     for dc in range(NDC)]
            wv_t = [persist.tile([128, O_V], fp16, name=f"wv{dc}", tag=f"wv{dc}")
                    for dc in range(NDC)]
            pw_t = [persist.tile([128, D], fp16, name=f"pw{dc}", tag=f"pw{dc}")
                    for dc in range(2)]
            xt_t = [persist.tile([128, S], fp16, name=f"xt{dc}", tag=f"xt{dc}")
                    for dc in range(NDC)]
            cos_t = persist.tile([128, S], fp16, name="cos", tag="cos")
            sin_t = persist.tile([128, S], fp16, name="sin", tag="sin")
            bqk_t = persist.tile([128, 4], fp32, name="bqk", tag="bqk")
            triA_t = persist.tile([128, 128], fp16, name="triA", tag="triA")
            negI_t = persist.tile([128, 128], fp16, name="negI", tag="negI")
            scratch = persist.tile([1, 2], fp32, name="scratch", tag="scratch")
            qk_t = [persist.tile([128, S], fp16, name=f"qk{ot}", tag=f"qk{ot}")
                    for ot in range(4)]
            vaug_t = [persist.tile([128, HPG * (HD + 1)], fp16, name=f"va{kt}", tag=f"va{kt}")
                      for kt in range(NKT)]
            outT_t = [persist.tile([128, S], fp16, name=f"oT{t}", tag=f"oT{t}")
                      for t in range(2)]

            # warm the Exp activation table while DMAs stream
            nc.gpsimd.memset(scratch[:], 0.0)
            nc.scalar.activation(scratch[:], scratch[:], Exp)

            # --- loads: spread across the SP / Act / DVE HWDGE rings so the
            # per-DMA fixed issue cost (~625ns) parallelizes; order by the
            # time each tensor is first needed.
            def dma_x(eng, st, dc):
                eng.dma_start(
                    out=xt_t[dc][:, st * 512:(st + 1) * 512],
                    in_=xT[dc * 128:(dc + 1) * 128, st * 512:(st + 1) * 512])

            # small early tensors on the gpsimd SWDGE queue (ready ~4us)
            nc.gpsimd.dma_start(out=bqk_t[:], in_=bqk)
            nc.gpsimd.dma_start(out=sin_t[:, 0:512], in_=sinF[:, 0:512])
            nc.gpsimd.dma_start(out=cos_t[:, 0:512], in_=cosF[:, 0:512])
            nc.gpsimd.dma_start(out=triA_t[:], in_=triA)
            nc.gpsimd.dma_start(out=negI_t[:], in_=negI)
            # first s-tile x + qk weights: weight chunk on one ring, its x
            # chunk on the other, so the dc=0 pair lands in parallel
            for dc in range(NDC):
                ea, eb = (nc.sync, nc.scalar) if dc % 2 == 0 else (nc.scalar, nc.sync)
                ea.dma_start(out=wqk_t[dc][:],
                             in_=wqkT[dc * 128:(dc + 1) * 128, :])
                dma_x(eb, 0, dc)
            # everything below streams during compute: keep it OFF the Act
            # ring (the Act sequencer must stay free for the softmax exps)
            for dc in range(NDC):
                eng = nc.gpsimd if dc % 2 == 0 else nc.sync
                eng.dma_start(out=wv_t[dc][:],
                              in_=wvT[dc * 128:(dc + 1) * 128, :])
            for dc in range(NDC):
                dma_x(nc.sync, 1, dc)
            nc.gpsimd.dma_start(out=pw_t[0][:], in_=pwT[0:128, :])
            nc.gpsimd.dma_start(out=pw_t[1][:], in_=pwT[128:256, :])
            nc.sync.dma_start(out=sin_t[:, 512:S], in_=sinF[:, 512:S])
            nc.sync.dma_start(out=cos_t[:, 512:S], in_=cosF[:, 512:S])
            for dc in range(NDC):
                nc.sync.dma_start(out=xt_t[dc][:, 1024:2048],
                                  in_=xT[dc * 128:(dc + 1) * 128, 1024:2048])
            for kt in range(NKT):
                nc.gpsimd.memset(vaug_t[kt][:], 1.0)

            # --- phase-1 work, chunked for interleave -----------------------
            def rope_from(ps_ap, st, ot):
                """RoPE tail: bias + rotate ps_ap -> qk_t[ot][:, st*512:...]."""
                s0 = st * 512
                tb = p1sb.tile([128, 512], fp16, name=f"tb{st}_{ot}", tag="tb")
                nc.vector.tensor_scalar_add(tb[:], ps_ap, bqk_t[:, ot:ot + 1])
                m2s = p1sb.tile([128, 512], fp16, name=f"m2s{st}_{ot}", tag="m2s")
                nc.vector.tensor_mul(m2s[:], tb[:], sin_t[:, s0:s0 + 512])
                m2 = p1sb.tile([128, 512], fp16, name=f"m2{st}_{ot}", tag="m2")
                for band in range(4):
                    dstp = band * 32
                    srcp = (band ^ 1) * 32
                    nc.vector.tensor_copy(m2[dstp:dstp + 32, :], m2s[srcp:srcp + 32, :])
                m1 = p1sb.tile([128, 512], fp16, name=f"m1{st}_{ot}", tag="m2s")
                nc.vector.tensor_mul(m1[:], tb[:], cos_t[:, s0:s0 + 512])
                nc.vector.tensor_add(qk_t[ot][:, s0:s0 + 512], m1[:], m2[:])

            def emit_qk(st, ot):
                """One (o 128, s 512) q^T/k^T tile: matmul + bias + RoPE."""
                s0 = st * 512
                ps = ps1_pool.tile([128, 512], fp32, name=f"psA{st}_{ot}", tag="ps1")
                for dc in range(NDC):
                    nc.tensor.matmul(ps[:], wqk_t[dc][:, ot * 128:(ot + 1) * 128],
                                     xt_t[dc][:, s0:s0 + 512],
                                     start=(dc == 0), stop=(dc == NDC - 1))
                rope_from(ps[:], st, ot)

            def phase1_start():
                """dc-outer qkv for s-tile 0: consume x chunks as they arrive.
                Uses the (idle at startup) psS pool as 4 parallel accumulators.
                psW[0] holds hp0's pair (ot0|ot2) so its slot frees early."""
                psW = [psS_pool.tile([128, 1024], fp32, name=f"psW{i}", tag="psS")
                       for i in range(2)]
                slot = {0: (0, 0), 2: (0, 512), 1: (1, 0), 3: (1, 512)}
                for dc in range(NDC):
                    for ot in (0, 2):
                        w, c0 = slot[ot]
                        nc.tensor.matmul(
                            psW[w][:, c0:c0 + 512],
                            wqk_t[dc][:, ot * 128:(ot + 1) * 128],
                            xt_t[dc][:, 0:512],
                            start=(dc == 0), stop=(dc == NDC - 1))
                # hp0's pair is complete: rope it while ot1/ot3 accumulate
                for ot in (0, 2):
                    w, c0 = slot[ot]
                    rope_from(psW[w][:, c0:c0 + 512], 0, ot)
                for dc in range(NDC):
                    for ot in (1, 3):
                        w, c0 = slot[ot]
                        nc.tensor.matmul(
                            psW[w][:, c0:c0 + 512],
                            wqk_t[dc][:, ot * 128:(ot + 1) * 128],
                            xt_t[dc][:, 0:512],
                            start=(dc == 0), stop=(dc == NDC - 1))
                for ss in range(4):
                    emit_v(0, ss)
                for ot in (1, 3):
                    w, c0 = slot[ot]
                    rope_from(psW[w][:, c0:c0 + 512], 0, ot)

            def emit_v(st, ss):
                """One (s 128, o 256) natural-layout V tile into vaug."""
                ps = ps1_pool.tile([128, O_V], fp32, name=f"psB{st}_{ss}", tag="ps1")
                s128 = st * 4 + ss
                for dc in range(NDC):
                    nc.tensor.matmul(ps[:], xt_t[dc][:, s128 * 128:(s128 + 1) * 128],
                                     wv_t[dc][:], start=(dc == 0), stop=(dc == NDC - 1))
                vt = vaug_t[s128]
                nc.vector.tensor_copy(
                    vt.rearrange("p (h w) -> p h w", w=HD + 1)[:, :, 0:HD],
                    ps.rearrange("p (h w) -> p h w", w=HD))

            def phase1_chunks(st):
                ch = []
                for ot in (0, 2, 1, 3):
                    ch.append(lambda ot=ot: emit_qk(st, ot))
                for ss in range(4):
                    ch.append(lambda ss=ss: emit_v(st, ss))
                return ch

            # --- attention for one q-tile, software-pipelined ---------------
            def attention(qt, filler):
                """filler: list of phase-1 chunk closures to interleave."""
                q0 = qt * 512
                kmax = (q0 + 512) // 128
                n_steps = 2 * kmax
                usable = max(0, len(filler) - RESERVE)
                fill_every = max(1, n_steps // usable) if usable else 0
                step = 0
                fired = 0

                def maybe_fill():
                    nonlocal step, fired
                    step += 1
                    if (filler and fill_every and fired < usable
                            and step % fill_every == 0):
                        fired += 1
                        filler.pop(0)()

                if qt == NST - 1:
                    # alternate long (non-crossing) and short (crossing) kts so
                    # the exp round-trip latency hides behind the long matmuls
                    cross = list(range(kmax - 4, kmax))
                    noncross = list(range(kmax - 4))
                    kseq = [noncross[0]]
                    rest = noncross[1:]
                    for i, c in enumerate(cross):
                        if i < len(rest):
                            kseq.append(rest[i])
                        kseq.append(c)
                    kseq += rest[len(cross):]
                else:
                    kseq = list(range(kmax))
                for hp in range(2):
                    q_tile = qk_t[hp]
                    k_tile = qk_t[2 + hp]
                    hA, hB = 2 * hp, 2 * hp + 1
                    psO_A = psX_pool.tile([128, 512], fp32, name=f"psOA{hp}_{qt}", tag="psX")
                    psO_B = psX_pool.tile([128, 512], fp32, name=f"psOB{hp}_{qt}", tag="psX")
                    e_tiles = [None] * kmax
                    assert kseq[0] == 0 or qt == 0  # first PV must cover all cols

                    def emit_st(kt):
                        k0 = kt * 128
                        jj = (k0 - q0) // 128  # >= 0 means diagonal-crossing
                        psS = psS_pool.tile([128, 1024], fp32, name=f"psS{hp}_{qt}_{kt}", tag="psS")
                        e = p2sb.tile([128, 1024], fp16, name=f"e{hp}_{qt}_{kt}", tag="e")
                        if jj < 0:
                            nc.tensor.matmul(psS[:, 0:512], k_tile[0:64, k0:k0 + 128],
                                             q_tile[0:64, q0:q0 + 512], start=True, stop=True,
                                             tile_position=(0, 0))
                            nc.tensor.matmul(psS[:, 512:1024], k_tile[64:128, k0:k0 + 128],
                                             q_tile[64:128, q0:q0 + 512], start=True, stop=True,
                                             tile_position=(64, 0))
                            if qt == NST - 1 and SPLIT_EXP_QT3:
                                # no fillers in the last q-tile: split the exp
                                # per head so each PV waits on half the latency
                                nc.scalar.activation(e[:, 0:512], psS[:, 0:512],
                                                     Exp, scale=float(SCALE))
                                nc.scalar.activation(e[:, 512:1024], psS[:, 512:1024],
                                                     Exp, scale=float(SCALE))
                            else:
                                nc.scalar.activation(e[:], psS[:], Exp, scale=float(SCALE))
                        else:
                            lo = jj * 128
                            # valid q range in one matmul per head; start=True
                            # resets the whole bank (zeroing the unwritten part).
                            # Head A fully (scores+mask) before head B so exp_A
                            # can start while B's scores run.
                            nc.tensor.matmul(psS[:, lo:512],
                                             k_tile[0:64, k0:k0 + 128],
                                             q_tile[0:64, q0 + lo:q0 + 512],
                                             start=True, stop=False, tile_position=(0, 0),
                                             skip_group_check=True)
                            nc.tensor.matmul(psS[:, lo:lo + 128], triA_t[:], negI_t[:],
                                             start=False, stop=True,
                                             skip_group_check=True)
                            nc.tensor.matmul(psS[:, 512 + lo:1024],
                                             k_tile[64:128, k0:k0 + 128],
                                             q_tile[64:128, q0 + lo:q0 + 512],
                                             start=True, stop=False, tile_position=(64, 0),
                                             skip_group_check=True)
                            nc.tensor.matmul(psS[:, 512 + lo:512 + lo + 128],
                                             triA_t[:], negI_t[:],
                                             start=False, stop=True,
                                             skip_group_check=True)
                            if SPLIT_EXP_CROSS:
                                nc.scalar.activation(e[:, lo:512], psS[:, lo:512],
                                                     Exp, scale=float(SCALE))
                                nc.scalar.activation(e[:, 512 + lo:1024],
                                                     psS[:, 512 + lo:1024],
                                                     Exp, scale=float(SCALE))
                            else:
                                pv = psS.rearrange("p (h q) -> p h q", q=512)[:, :, lo:512]
                                ev = e.rearrange("p (h q) -> p h q", q=512)[:, :, lo:512]
                                nc.scalar.activation(ev, pv, Exp, scale=float(SCALE))
                        e_tiles[kt] = e

                    def emit_pv(kt, st_f, sp_f):
                        k0 = kt * 128
                        jj = (k0 - q0) // 128
                        lo = jj * 128 if jj > 0 else 0
                        e = e_tiles[kt]
                        nc.tensor.matmul(psO_A[0:HD + 1, lo:512],
                                         vaug_t[kt][:, hA * 65:hA * 65 + 65],
                                         e[:, lo:512], start=st_f, stop=sp_f,
                                         skip_group_check=True)
                        nc.tensor.matmul(psO_B[0:HD + 1, lo:512],
                                         vaug_t[kt][:, hB * 65:hB * 65 + 65],
                                         e[:, 512 + lo:1024], start=st_f, stop=sp_f,
                                         skip_group_check=True)

                    emit_st(kseq[0])
                    for i, kt in enumerate(kseq):
                        if i + 1 < kmax:
                            emit_st(kseq[i + 1])
                        emit_pv(kt, st_f=(i == 0), sp_f=(i == kmax - 1))
                        maybe_fill()

                    for idx, psO in ((0, psO_A), (1, psO_B)):
                        rec = nrm.tile([1, 512], fp32, name=f"rec{hp}_{qt}_{idx}", tag="rec")
                        nc.vector.reciprocal(rec[0:1, :], psO[HD:HD + 1, :])
                        rbc = nrm.tile([64, 512], fp32, name=f"rbc{hp}_{qt}_{idx}", tag="rbc")
                        nc.gpsimd.partition_broadcast(rbc[:], rec[0:1, :])
                        nc.vector.tensor_mul(
                            outT_t[hp][idx * 64:(idx + 1) * 64, q0:q0 + 512],
                            psO[0:64, :], rbc[:])
                while filler:
                    filler.pop(0)()

            # --- output projection for one q-tile ---------------------------
            def proj(qt, ssts=(0, 1, 2, 3), pool=None):
                for sst in ssts:
                    st128 = qt * 4 + sst
                    po = p3sb.tile([128, 1024], fp16, name=f"po{st128}", tag="po")
                    pp = pool or psX_pool
                    for jt in range(2):
                        ps = pp.tile([128, 512], fp32, name=f"psP{st128}_{jt}",
                                     tag="ps1" if pool else "psX")
                        for dc in range(2):
                            nc.tensor.matmul(ps[:], outT_t[dc][:, st128 * 128:(st128 + 1) * 128],
                                             pw_t[dc][:, jt * 512:(jt + 1) * 512],
                                             start=(dc == 0), stop=(dc == 1))
                        if jt == 0:
                            nc.vector.tensor_copy(po[:, 0:512], ps[:])
                        elif pool is not None:
                            nc.vector.tensor_copy(po[:, 512:1024], ps[:])
                        else:
                            nc.scalar.copy(po[:, 512:1024], ps[:])
                    # spread output DMAs across all three queues so the final
                    # drain parallelizes (inputs are done by the time these go)
                    if qt == NST - 1 and sst == 3:
                        nc.sync.dma_start(
                            out=outP[st128 * 128:(st128 + 1) * 128, 0:512],
                            in_=po[:, 0:512])
                        nc.gpsimd.dma_start(
                            out=outP[st128 * 128:(st128 + 1) * 128, 512:1024],
                            in_=po[:, 512:1024])
                    else:
                        eng = (nc.gpsimd, nc.sync, nc.gpsimd, nc.sync)[sst]
                        eng.dma_start(
                            out=outP[st128 * 128:(st128 + 1) * 128, :], in_=po[:])

            # --- main schedule ----------------------------------------------
            phase1_start()
            for qt in range(NST):
                if qt + 1 < NST:
                    filler = phase1_chunks(qt + 1)
                elif qt == NST - 1:
                    # the last attention has no phase-1 left: feed it the
                    # deferred proj(1)/proj(2) blocks (ps1 pool is idle by then)
                    filler = [lambda s=s: proj(2, ssts=(s,), pool=ps1_pool)
                              for s in range(4)]
                    filler += [lambda s=s: proj(1, ssts=(s,), pool=ps1_pool)
                               for s in (2, 3)]
                attention(qt, filler)
                if qt == 1:
                    proj(qt, ssts=(0, 1))
                elif qt != 2:
                    proj(qt)

    nc.compile()
    return nc


def _get_nc():
    global _NC_CACHE
    if _NC_CACHE is None:
        _NC_CACHE = _build_nc()
    return _NC_CACHE


def _prep_in_maps(x, qkv_w, qkv_b, proj_w, proj_b):
    x = np.asarray(x, dtype=np.float32)
    qkv_w = np.asarray(qkv_w, dtype=np.float32)
    qkv_b = np.asarray(qkv_b, dtype=np.float32)
    proj_w = np.asarray(proj_w, dtype=np.float32)

    # RoPE tables (match reference fp32 math, then cast to fp16)
    freqs = (1.0 / (10000.0 ** (np.arange(HALF, dtype=np.float32) / HALF))).astype(np.float32)
    pos = np.arange(S, dtype=np.float32)
    ang = pos[:, None] * freqs[None, :]            # (S, 32)
    cos_m = np.cos(ang).astype(np.float32)
    sin_m = np.sin(ang).astype(np.float32)
    p = np.arange(128)
    cosF = np.ascontiguousarray(cos_m[:, p % HALF].T).astype(np.float16)   # (128, S)
    sign = np.where((p % HD) < HALF, 1.0, -1.0).astype(np.float32)
    sinF = np.ascontiguousarray((sin_m[:, p % HALF] * sign[None, :]).T).astype(np.float16)

    # causal mask via PE: (triA.T @ negI)[p, f] = -60000 * (f < p)
    kk, pp2 = np.meshgrid(np.arange(128), np.arange(128), indexing="ij")
    triA = (pp2 > kk).astype(np.float16)             # [k, p] = 1 where p > k
    negI = (-60000.0 * np.eye(128)).astype(np.float16)

    in_maps = []
    for c in range(NCORES):
        b, g = divmod(c, GROUPS)
        hs = np.arange(HPG) + HPG * g
        q_rows = np.concatenate([np.arange(HD * h, HD * h + HD) for h in hs])
        qk_rows = np.concatenate([q_rows, q_rows + D])
        in_maps.append({
            "xT": np.ascontiguousarray(x[b].T).astype(np.float16),
            "wqkT": np.ascontiguousarray(qkv_w[qk_rows, :].T).astype(np.float16),
            "wvT": np.ascontiguousarray(qkv_w[q_rows + 2 * D, :].T).astype(np.float16),
            "bqk": np.ascontiguousarray(qkv_b[qk_rows].reshape(4, 128).T),
            "cosF": cosF,
            "sinF": sinF,
            "pwT": np.ascontiguousarray(proj_w[:, q_rows].T).astype(np.float16),
            "triA": triA,
            "negI": negI,
        })
    return in_maps


def _combine(results, qkv_b, proj_w, proj_b):
    """Sum per-core partials; add proj bias and the folded V-bias term."""
    qkv_b = np.asarray(qkv_b, dtype=np.float32)
    proj_w = np.asarray(proj_w, dtype=np.float32)
    proj_b = np.asarray(proj_b, dtype=np.float32)
    bv_fold = proj_w @ qkv_b[2 * D:3 * D]          # (D,)
    const = (proj_b + bv_fold).astype(np.float64)
    out = np.empty((B, S, D), dtype=np.float32)
    for b in range(B):
        acc = np.zeros((S, D), dtype=np.float64)
        for g in range(GROUPS):
            acc += results[b * GROUPS + g]["out_partial"].astype(np.float64)
        out[b] = (acc + const[None, :]).astype(np.float32)
    return out


def _run(x, qkv_w, qkv_b, proj_w, proj_b, trace=False):
    from concourse import bass_utils
    nc = _get_nc()
    in_maps = _prep_in_maps(x, qkv_w, qkv_b, proj_w, proj_b)
    res = bass_utils.run_bass_kernel_spmd(nc, in_maps, core_ids=list(range(NCORES)),
                                          trace=trace)
    out = _combine(res.results, qkv_b, proj_w, proj_b)
    return out, res


def kernel(x, qkv_w, qkv_b, proj_w, proj_b):
    out, _ = _run(x, qkv_w, qkv_b, proj_w, proj_b, trace=False)
    return out


# revision 6
# speedup vs baseline: 1.2449x; 1.2449x over previous
"""Multi-head causal attention (RoPE) for Trainium2, sharded over 8 NeuronCores.

Sharding: core c = 4*b + g handles batch b (of 2) and head-group g (4 of 16 heads).
Each core computes the qkv projection for its heads, RoPE, causal attention, and
a partial output projection (row-parallel over its heads' dims). The host sums
the 4 partials per batch and adds proj_b (plus the folded V-bias term).

v2 changes vs baseline:
  - fp16 operands everywhere on-chip (PSUM accumulation stays fp32): halves DMA
    and SBUF traffic, 2x DVE throughput on RoPE elementwise ops, and allows
    128-wide matmuls at 1 cycle/row (fp32r would be 4x for moving<256).
  - fine-grained causal diagonal: crossing k-tiles only compute the valid q
    range (start=True resets the whole PSUM bank, zeroing the rest), and the
    triangular 128x128 sub-block mask is injected by one extra 128-moving
    matmul: psS += triA.T @ (-60000*I) adds -60000 exactly where q < k, so
    exp() flushes those to zero. This removes the gpsimd affine_select from
    the exp->PV critical path entirely and keeps masking PE-local.
  - V bias folded into the host-side combine (softmax weights sum to 1, so the
    bias contributes bv @ proj_w.T, a constant row); removes the normalize-time
    bias adds.
  - x is fully SBUF-resident; the qkv weights and the first x s-tile are
    host-packed into one DRAM tensor (8 fused startup DMAs), input DMAs are
    spread across the SP/Act HWDGE rings + gpsimd SWDGE queue by need-time,
    and the first s-tile's qkv runs dc-outer so it consumes chunks as they
    arrive.
  - proj(0)/proj(1)/proj(2) are partially deferred into the (filler-less)
    last attention's kt-loop, the last normalize window is covered by the
    reserved chunks, and the last q-tile's output DMAs fire per 512-column
    half across three queues as soon as each PSUM->SBUF copy lands.
  - proj PSUM shares a pool with the attention output accumulators (disjoint
    lifetimes), proj output is staged as one [128, 1024] tile per 128-row
    block -> one output DMA per block.
"""
import numpy as np

B, S, D = 2, 2048, 1024
HEADS, HD = 16, 64
HALF = HD // 2
NCORES = 8
GROUPS = 4          # tensor-parallel head groups per batch
HPG = HEADS // GROUPS
O_QK = 2 * HPG * HD   # 512 rows: [q h0..h3 | k h0..h3]
O_V = HPG * HD        # 256
SCALE = 1.0 / np.sqrt(HD)

NST = S // 512        # 4 seq tiles of 512
NDC = D // 128        # 8 contraction chunks
NKT = S // 128        # 16 k tiles

_NC_CACHE = None

# schedule knobs
PSS_BUFS = 2
E_BUFS = 7
RESERVE = 2   # fillers kept back to cover the normalize window
P1_BUFS = 3
NRM_BUFS = 2
SPLIT_EXP_QT3 = False
SPLIT_EXP_CROSS = False


def _build_nc():
    import concourse.bacc as bacc
    import concourse.mybir as mybir
    import concourse.tile as tile

    fp32 = mybir.dt.float32
    fp16 = mybir.dt.float16
    Exp = mybir.ActivationFunctionType.Exp

    nc = bacc.Bacc(trn_type="TRN2", target_bir_lowering=False, debug=False)

    xT = nc.dram_tensor("xT", [D, S], fp16, kind="ExternalInput").ap()
    wx0 = nc.dram_tensor("wx0", [D, O_QK + 512], fp16, kind="ExternalInput").ap()
    wvT = nc.dram_tensor("wvT", [D, O_V], fp16, kind="ExternalInput").ap()
    bqk = nc.dram_tensor("bqk", [128, 4], fp32, kind="ExternalInput").ap()
    cosF = nc.dram_tensor("cosF", [128, S], fp16, kind="ExternalInput").ap()
    sinF = nc.dram_tensor("sinF", [128, S], fp16, kind="ExternalInput").ap()
    pwT = nc.dram_tensor("pwT", [O_V, D], fp16, kind="ExternalInput").ap()
    triA = nc.dram_tensor("triA", [128, 128], fp16, kind="ExternalInput").ap()
    negI = nc.dram_tensor("negI", [128, 128], fp16, kind="ExternalInput").ap()
    outP = nc.dram_tensor("out_partial", [S, D], fp16, kind="ExternalOutput").ap()

    with tile.TileContext(nc) as tc:
        with tc.tile_pool(name="persist", bufs=1) as persist, \
             tc.tile_pool(name="p1sb", bufs=P1_BUFS) as p1sb, \
             tc.tile_pool(name="p2sb", bufs=E_BUFS) as p2sb, \
             tc.tile_pool(name="nrm", bufs=4) as nrm, \
             tc.tile_pool(name="p3sb", bufs=4) as p3sb, \
             tc.tile_pool(name="ps1", bufs=2, space="PSUM") as ps1_pool, \
             tc.tile_pool(name="psS", bufs=PSS_BUFS, space="PSUM") as psS_pool, \
             tc.tile_pool(name="psX", bufs=2, space="PSUM") as psX_pool:

            # --- persistent tiles -------------------------------------------
            # packed [wqk | x s-tile 0] per dc chunk: one startup DMA each
            wx0_t = [persist.tile([128, O_QK + 512], fp16, name=f"wx0{dc}", tag=f"wx0{dc}")
                     for dc in range(NDC)]
            wv_t = [persist.tile([128, O_V], fp16, name=f"wv{dc}", tag=f"wv{dc}")
                    for dc in range(NDC)]
            pw_t = [persist.tile([128, D], fp16, name=f"pw{dc}", tag=f"pw{dc}")
                    for dc in range(2)]
            xt_t = [persist.tile([128, S], fp16, name=f"xt{dc}", tag=f"xt{dc}")
                    for dc in range(NDC)]
            cos_t = persist.tile([128, S], fp16, name="cos", tag="cos")
            sin_t = persist.tile([128, S], fp16, name="sin", tag="sin")
            bqk_t = persist.tile([128, 4], fp32, name="bqk", tag="bqk")
            triA_t = persist.tile([128, 128], fp16, name="triA", tag="triA")
            negI_t = persist.tile([128, 128], fp16, name="negI", tag="negI")
            scratch = persist.tile([1, 2], fp32, name="scratch", tag="scratch")
            qk_t = [persist.tile([128, S], fp16, name=f"qk{ot}", tag=f"qk{ot}")
                    for ot in range(4)]
            vaug_t = [persist.tile([128, HPG * (HD + 1)], fp16, name=f"va{kt}", tag=f"va{kt}")
                      for kt in range(NKT)]
            outT_t = [persist.tile([128, S], fp16, name=f"oT{t}", tag=f"oT{t}")
                      for t in range(2)]

            # warm the Exp activation table while DMAs stream
            nc.gpsimd.memset(scratch[:], 0.0)
            nc.scalar.activation(scratch[:], scratch[:], Exp)

            # --- loads: spread across the SP / Act / DVE HWDGE rings so the
            # per-DMA fixed issue cost (~625ns) parallelizes; order by the
            # time each tensor is first needed.
            def dma_x(eng, st, dc):
                eng.dma_start(
                    out=xt_t[dc][:, st * 512:(st + 1) * 512],
                    in_=xT[dc * 128:(dc + 1) * 128, st * 512:(st + 1) * 512])

            # small early tensors on the gpsimd SWDGE queue (ready ~4us)
            nc.gpsimd.dma_start(out=bqk_t[:], in_=bqk)
            nc.gpsimd.dma_start(out=sin_t[:, 0:512], in_=sinF[:, 0:512])
            nc.gpsimd.dma_start(out=cos_t[:, 0:512], in_=cosF[:, 0:512])
            nc.gpsimd.dma_start(out=triA_t[:], in_=triA)
            nc.gpsimd.dma_start(out=negI_t[:], in_=negI)
            # packed qk-weights + first-s-tile x: big DMAs over both rings;
            # dc=0 is split in half across the rings so the first matmul's
            # inputs land as early as possible
            nc.sync.dma_start(out=wx0_t[0][:, 0:512], in_=wx0[0:128, 0:512])
            nc.scalar.dma_start(out=wx0_t[0][:, 512:1024], in_=wx0[0:128, 512:1024])
            for dc in range(1, NDC):
                eng = nc.sync if dc % 2 == 0 else nc.scalar
                eng.dma_start(out=wx0_t[dc][:],
                              in_=wx0[dc * 128:(dc + 1) * 128, :])
            # everything below streams during compute: keep it OFF the Act
            # ring (the Act sequencer must stay free for the softmax exps)
            for dc in range(NDC):
                eng = nc.gpsimd if dc % 2 == 0 else nc.sync
                eng.dma_start(out=wv_t[dc][:],
                              in_=wvT[dc * 128:(dc + 1) * 128, :])
            for dc in range(NDC):
                dma_x(nc.sync, 1, dc)
            nc.gpsimd.dma_start(out=pw_t[0][:], in_=pwT[0:128, :])
            nc.gpsimd.dma_start(out=pw_t[1][:], in_=pwT[128:256, :])
            nc.sync.dma_start(out=sin_t[:, 512:S], in_=sinF[:, 512:S])
            nc.sync.dma_start(out=cos_t[:, 512:S], in_=cosF[:, 512:S])
            for dc in range(NDC):
                nc.sync.dma_start(out=xt_t[dc][:, 1024:2048],
                                  in_=xT[dc * 128:(dc + 1) * 128, 1024:2048])
            for kt in range(NKT):
                nc.gpsimd.memset(vaug_t[kt][:], 1.0)

            def wqk_ap(dc, c0, c1):
                return wx0_t[dc][:, c0:c1]

            def x_ap(dc, e0, e1):
                # x element range [e0:e1) of the full row; s-tile 0 is packed
                if e1 <= 512:
                    return wx0_t[dc][:, O_QK + e0:O_QK + e1]
                return xt_t[dc][:, e0:e1]

            # --- phase-1 work, chunked for interleave -----------------------
            def rope_from(ps_ap, st, ot):
                """RoPE tail: bias + rotate ps_ap -> qk_t[ot][:, st*512:...]."""
                s0 = st * 512
                tb = p1sb.tile([128, 512], fp16, name=f"tb{st}_{ot}", tag="tb")
                nc.vector.tensor_scalar_add(tb[:], ps_ap, bqk_t[:, ot:ot + 1])
                m2s = p1sb.tile([128, 512], fp16, name=f"m2s{st}_{ot}", tag="m2s")
                nc.vector.tensor_mul(m2s[:], tb[:], sin_t[:, s0:s0 + 512])
                m2 = p1sb.tile([128, 512], fp16, name=f"m2{st}_{ot}", tag="m2")
                for band in range(4):
                    dstp = band * 32
                    srcp = (band ^ 1) * 32
                    nc.vector.tensor_copy(m2[dstp:dstp + 32, :], m2s[srcp:srcp + 32, :])
                m1 = p1sb.tile([128, 512], fp16, name=f"m1{st}_{ot}", tag="m2s")
                nc.vector.tensor_mul(m1[:], tb[:], cos_t[:, s0:s0 + 512])
                nc.vector.tensor_add(qk_t[ot][:, s0:s0 + 512], m1[:], m2[:])

            def emit_qk(st, ot):
                """One (o 128, s 512) q^T/k^T tile: matmul + bias + RoPE."""
                s0 = st * 512
                ps = ps1_pool.tile([128, 512], fp32, name=f"psA{st}_{ot}", tag="ps1")
                for dc in range(NDC):
                    nc.tensor.matmul(ps[:], wqk_ap(dc, ot * 128, (ot + 1) * 128),
                                     x_ap(dc, s0, s0 + 512),
                                     start=(dc == 0), stop=(dc == NDC - 1))
                rope_from(ps[:], st, ot)

            def phase1_start():
                """dc-outer qkv for s-tile 0: consume x chunks as they arrive.
                Uses the (idle at startup) psS pool as 4 parallel accumulators.
                psW[0] holds hp0's pair (ot0|ot2) so its slot frees early."""
                psW = [psS_pool.tile([128, 1024], fp32, name=f"psW{i}", tag="psS")
                       for i in range(2)]
                slot = {0: (0, 0), 2: (0, 512), 1: (1, 0), 3: (1, 512)}
                for dc in range(NDC):
                    for ot in (0, 2):
                        w, c0 = slot[ot]
                        nc.tensor.matmul(
                            psW[w][:, c0:c0 + 512],
                            wqk_ap(dc, ot * 128, (ot + 1) * 128),
                            x_ap(dc, 0, 512),
                            start=(dc == 0), stop=(dc == NDC - 1))
                # hp0's pair is complete: rope it while ot1/ot3 accumulate
                for ot in (0, 2):
                    w, c0 = slot[ot]
                    rope_from(psW[w][:, c0:c0 + 512], 0, ot)
                for dc in range(NDC):
                    for ot in (1, 3):
                        w, c0 = slot[ot]
                        nc.tensor.matmul(
                            psW[w][:, c0:c0 + 512],
                            wqk_ap(dc, ot * 128, (ot + 1) * 128),
                            x_ap(dc, 0, 512),
                            start=(dc == 0), stop=(dc == NDC - 1))
                for ss in range(4):
                    emit_v(0, ss)
                for ot in (1, 3):
                    w, c0 = slot[ot]
                    rope_from(psW[w][:, c0:c0 + 512], 0, ot)

            def emit_v(st, ss):
                """One (s 128, o 256) natural-layout V tile into vaug."""
                ps = ps1_pool.tile([128, O_V], fp32, name=f"psB{st}_{ss}", tag="ps1")
                s128 = st * 4 + ss
                for dc in range(NDC):
                    nc.tensor.matmul(ps[:], x_ap(dc, s128 * 128, (s128 + 1) * 128),
                                     wv_t[dc][:], start=(dc == 0), stop=(dc == NDC - 1))
                vt = vaug_t[s128]
                nc.vector.tensor_copy(
                    vt.rearrange("p (h w) -> p h w", w=HD + 1)[:, :, 0:HD],
                    ps.rearrange("p (h w) -> p h w", w=HD))

            def phase1_chunks(st):
                ch = []
                for ot in (0, 2, 1, 3):
                    ch.append(lambda ot=ot: emit_qk(st, ot))
                for ss in range(4):
                    ch.append(lambda ss=ss: emit_v(st, ss))
                return ch

            # --- attention for one q-tile, software-pipelined ---------------
            def attention(qt, filler, reserve=RESERVE):
                """filler: list of phase-1 chunk closures to interleave."""
                q0 = qt * 512
                kmax = (q0 + 512) // 128
                n_steps = 2 * kmax
                usable = max(0, len(filler) - reserve)
                fill_every = max(1, n_steps // usable) if usable else 0
                step = 0
                fired = 0

                def maybe_fill():
                    nonlocal step, fired
                    step += 1
                    if (filler and fill_every and fired < usable
                            and step % fill_every == 0):
                        fired += 1
                        filler.pop(0)()

                if qt == NST - 1:
                    # alternate long (non-crossing) and short (crossing) kts so
                    # the exp round-trip latency hides behind the long matmuls
                    cross = list(range(kmax - 4, kmax))
                    noncross = list(range(kmax - 4))
                    kseq = [noncross[0]]
                    rest = noncross[1:]
                    for i, c in enumerate(cross):
                        if i < len(rest):
                            kseq.append(rest[i])
                        kseq.append(c)
                    kseq += rest[len(cross):]
                else:
                    kseq = list(range(kmax))
                for hp in range(2):
                    q_tile = qk_t[hp]
                    k_tile = qk_t[2 + hp]
                    hA, hB = 2 * hp, 2 * hp + 1
                    psO_A = psX_pool.tile([128, 512], fp32, name=f"psOA{hp}_{qt}", tag="psX")
                    psO_B = psX_pool.tile([128, 512], fp32, name=f"psOB{hp}_{qt}", tag="psX")
                    e_tiles = [None] * kmax
                    assert kseq[0] == 0 or qt == 0  # first PV must cover all cols

                    def emit_st(kt):
                        k0 = kt * 128
                        jj = (k0 - q0) // 128  # >= 0 means diagonal-crossing
                        psS = psS_pool.tile([128, 1024], fp32, name=f"psS{hp}_{qt}_{kt}", tag="psS")
                        e = p2sb.tile([128, 1024], fp16, name=f"e{hp}_{qt}_{kt}", tag="e")
                        if jj < 0:
                            nc.tensor.matmul(psS[:, 0:512], k_tile[0:64, k0:k0 + 128],
                                             q_tile[0:64, q0:q0 + 512], start=True, stop=True,
                                             tile_position=(0, 0))
                            nc.tensor.matmul(psS[:, 512:1024], k_tile[64:128, k0:k0 + 128],
                                             q_tile[64:128, q0:q0 + 512], start=True, stop=True,
                                             tile_position=(64, 0))
                            if qt == NST - 1 and SPLIT_EXP_QT3:
                                # no fillers in the last q-tile: split the exp
                                # per head so each PV waits on half the latency
                                nc.scalar.activation(e[:, 0:512], psS[:, 0:512],
                                                     Exp, scale=float(SCALE))
                                nc.scalar.activation(e[:, 512:1024], psS[:, 512:1024],
                                                     Exp, scale=float(SCALE))
                            else:
                                nc.scalar.activation(e[:], psS[:], Exp, scale=float(SCALE))
                        else:
                            lo = jj * 128
                            # valid q range in one matmul per head; start=True
                            # resets the whole bank (zeroing the unwritten part).
                            # Head A fully (scores+mask) before head B so exp_A
                            # can start while B's scores run.
                            nc.tensor.matmul(psS[:, lo:512],
                                             k_tile[0:64, k0:k0 + 128],
                                             q_tile[0:64, q0 + lo:q0 + 512],
                                             start=True, stop=False, tile_position=(0, 0),
                                             skip_group_check=True)
                            nc.tensor.matmul(psS[:, lo:lo + 128], triA_t[:], negI_t[:],
                                             start=False, stop=True,
                                             skip_group_check=True)
                            nc.tensor.matmul(psS[:, 512 + lo:1024],
                                             k_tile[64:128, k0:k0 + 128],
                                             q_tile[64:128, q0 + lo:q0 + 512],
                                             start=True, stop=False, tile_position=(64, 0),
                                             skip_group_check=True)
                            nc.tensor.matmul(psS[:, 512 + lo:512 + lo + 128],
                                             triA_t[:], negI_t[:],
                                             start=False, stop=True,
                                             skip_group_check=True)
                            if SPLIT_EXP_CROSS:
                                nc.scalar.activation(e[:, lo:512], psS[:, lo:512],
                                                     Exp, scale=float(SCALE))
                                nc.scalar.activation(e[:, 512 + lo:1024],
                                                     psS[:, 512 + lo:1024],
                                                     Exp, scale=float(SCALE))
                            else:
                                pv = psS.rearrange("p (h q) -> p h q", q=512)[:, :, lo:512]
                                ev = e.rearrange("p (h q) -> p h q", q=512)[:, :, lo:512]
                                nc.scalar.activation(ev, pv, Exp, scale=float(SCALE))
                        e_tiles[kt] = e

                    def emit_pv(kt, st_f, sp_f):
                        k0 = kt * 128
                        jj = (k0 - q0) // 128
                        lo = jj * 128 if jj > 0 else 0
                        e = e_tiles[kt]
                        nc.tensor.matmul(psO_A[0:HD + 1, lo:512],
                                         vaug_t[kt][:, hA * 65:hA * 65 + 65],
                                         e[:, lo:512], start=st_f, stop=sp_f,
                                         skip_group_check=True)
                        nc.tensor.matmul(psO_B[0:HD + 1, lo:512],
                                         vaug_t[kt][:, hB * 65:hB * 65 + 65],
                                         e[:, 512 + lo:1024], start=st_f, stop=sp_f,
                                         skip_group_check=True)

                    emit_st(kseq[0])
                    for i, kt in enumerate(kseq):
                        if i + 1 < kmax:
                            emit_st(kseq[i + 1])
                        emit_pv(kt, st_f=(i == 0), sp_f=(i == kmax - 1))
                        maybe_fill()

                    for idx, psO in ((0, psO_A), (1, psO_B)):
                        rec = nrm.tile([1, 512], fp32, name=f"rec{hp}_{qt}_{idx}", tag="rec")
                        nc.vector.reciprocal(rec[0:1, :], psO[HD:HD + 1, :])
                        rbc = nrm.tile([64, 512], fp32, name=f"rbc{hp}_{qt}_{idx}", tag="rbc")
                        nc.gpsimd.partition_broadcast(rbc[:], rec[0:1, :])
                        nc.vector.tensor_mul(
                            outT_t[hp][idx * 64:(idx + 1) * 64, q0:q0 + 512],
                            psO[0:64, :], rbc[:])
                while filler:
                    filler.pop(0)()

            # --- output projection for one q-tile ---------------------------
            def proj(qt, ssts=(0, 1, 2, 3), pool=None):
                for sst in ssts:
                    st128 = qt * 4 + sst
                    po = p3sb.tile([128, 1024], fp16, name=f"po{st128}", tag="po")
                    pp = pool or psX_pool
                    for jt in range(2):
                        ps = pp.tile([128, 512], fp32, name=f"psP{st128}_{jt}",
                                     tag="ps1" if pool else "psX")
                        for dc in range(2):
                            nc.tensor.matmul(ps[:], outT_t[dc][:, st128 * 128:(st128 + 1) * 128],
                                             pw_t[dc][:, jt * 512:(jt + 1) * 512],
                                             start=(dc == 0), stop=(dc == 1))
                        if jt == 0:
                            nc.vector.tensor_copy(po[:, 0:512], ps[:])
                        elif pool is not None:
                            nc.vector.tensor_copy(po[:, 512:1024], ps[:])
                        else:
                            nc.scalar.copy(po[:, 512:1024], ps[:])
                        if qt == NST - 1:
                            # fire each half's DMA as soon as its copy lands
                            eng = ((nc.sync, nc.gpsimd, nc.sync, nc.gpsimd),
                                   (nc.gpsimd, nc.scalar, nc.scalar, nc.sync))[jt][sst]
                            eng.dma_start(
                                out=outP[st128 * 128:(st128 + 1) * 128,
                                         jt * 512:(jt + 1) * 512],
                                in_=po[:, jt * 512:(jt + 1) * 512])
                    # spread output DMAs across all three queues so the final
                    # drain parallelizes (inputs are done by the time these go)
                    if qt != NST - 1:
                        eng = (nc.gpsimd, nc.sync, nc.gpsimd, nc.sync)[sst]
                        eng.dma_start(
                            out=outP[st128 * 128:(st128 + 1) * 128, :], in_=po[:])

            # --- main schedule ----------------------------------------------
            phase1_start()
            for qt in range(NST):
                if qt + 1 < NST:
                    filler = phase1_chunks(qt + 1)
                elif qt == NST - 1:
                    # the last attention has no phase-1 left: feed it the
                    # deferred proj(1)/proj(2) blocks (ps1 pool is idle by then)
                    filler = [lambda s=s: proj(2, ssts=(s,), pool=ps1_pool)
                              for s in range(4)]
                    filler += [lambda s=s: proj(1, ssts=(s,), pool=ps1_pool)
                               for s in (1, 2, 3)]
                    filler += [lambda s=s: proj(0, ssts=(s,), pool=ps1_pool)
                               for s in (2, 3)]
                attention(qt, filler, reserve=5 if qt == NST - 1 else RESERVE)
                if qt == 0:
                    proj(qt, ssts=(0, 1))
                elif qt == 1:
                    proj(qt, ssts=(0,))
                elif qt != 2:
                    proj(qt)

    nc.compile()
    return nc


def _get_nc():
    global _NC_CACHE
    if _NC_CACHE is None:
        _NC_CACHE = _build_nc()
    return _NC_CACHE


def _prep_in_maps(x, qkv_w, qkv_b, proj_w, proj_b):
    x = np.asarray(x, dtype=np.float32)
    qkv_w = np.asarray(qkv_w, dtype=np.float32)
    qkv_b = np.asarray(qkv_b, dtype=np.float32)
    proj_w = np.asarray(proj_w, dtype=np.float32)

    # RoPE tables (match reference fp32 math, then cast to fp16)
    freqs = (1.0 / (10000.0 ** (np.arange(HALF, dtype=np.float32) / HALF))).astype(np.float32)
    pos = np.arange(S, dtype=np.float32)
    ang = pos[:, None] * freqs[None, :]            # (S, 32)
    cos_m = np.cos(ang).astype(np.float32)
    sin_m = np.sin(ang).astype(np.float32)
    p = np.arange(128)
    cosF = np.ascontiguousarray(cos_m[:, p % HALF].T).astype(np.float16)   # (128, S)
    sign = np.where((p % HD) < HALF, 1.0, -1.0).astype(np.float32)
    sinF = np.ascontiguousarray((sin_m[:, p % HALF] * sign[None, :]).T).astype(np.float16)

    # causal mask via PE: (triA.T @ negI)[p, f] = -60000 * (f < p)
    kk, pp2 = np.meshgrid(np.arange(128), np.arange(128), indexing="ij")
    triA = (pp2 > kk).astype(np.float16)             # [k, p] = 1 where p > k
    negI = (-60000.0 * np.eye(128)).astype(np.float16)

    in_maps = []
    for c in range(NCORES):
        b, g = divmod(c, GROUPS)
        hs = np.arange(HPG) + HPG * g
        q_rows = np.concatenate([np.arange(HD * h, HD * h + HD) for h in hs])
        qk_rows = np.concatenate([q_rows, q_rows + D])
        in_maps.append({
            "xT": np.ascontiguousarray(x[b].T).astype(np.float16),
            "wx0": np.ascontiguousarray(
                np.concatenate([qkv_w[qk_rows, :].T, x[b].T[:, 0:512]], axis=1)
            ).astype(np.float16),
            "wvT": np.ascontiguousarray(qkv_w[q_rows + 2 * D, :].T).astype(np.float16),
            "bqk": np.ascontiguousarray(qkv_b[qk_rows].reshape(4, 128).T),
            "cosF": cosF,
            "sinF": sinF,
            "pwT": np.ascontiguousarray(proj_w[:, q_rows].T).astype(np.float16),
            "triA": triA,
            "negI": negI,
        })
    return in_maps


def _combine(results, qkv_b, proj_w, proj_b):
    """Sum per-core partials; add proj bias and the folded V-bias term."""
    qkv_b = np.asarray(qkv_b, dtype=np.float32)
    proj_w = np.asarray(proj_w, dtype=np.float32)
    proj_b = np.asarray(proj_b, dtype=np.float32)
    bv_fold = proj_w @ qkv_b[2 * D:3 * D]          # (D,)
    const = (proj_b + bv_fold).astype(np.float64)
    out = np.empty((B, S, D), dtype=np.float32)
    for b in range(B):
        acc = np.zeros((S, D), dtype=np.float64)
        for g in range(GROUPS):
            acc += results[b * GROUPS + g]["out_partial"].astype(np.float64)
        out[b] = (acc + const[None, :]).astype(np.float32)
    return out


def _run(x, qkv_w, qkv_b, proj_w, proj_b, trace=False):
    from concourse import bass_utils
    nc = _get_nc()
    in_maps = _prep_in_maps(x, qkv_w, qkv_b, proj_w, proj_b)
    res = bass_utils.run_bass_kernel_spmd(nc, in_maps, core_ids=list(range(NCORES)),
                                          trace=trace)
    out = _combine(res.results, qkv_b, proj_w, proj_b)
    return out, res


def kernel(x, qkv_w, qkv_b, proj_w, proj_b):
    out, _ = _run(x, qkv_w, qkv_b, proj_w, proj_b, trace=False)
    return out


# revision 7
# speedup vs baseline: 37.1445x; 29.8370x over previous
"""Multi-head causal attention (RoPE) for Trainium2, sharded over 8 NeuronCores.

Sharding: core c = 4*b + g handles batch b (of 2) and head-group g (4 of 16 heads).
Each core computes the qkv projection for its heads, RoPE, causal attention, and
a partial output projection (row-parallel over its heads' dims). The host sums
the 4 partials per batch and adds proj_b (plus the folded V-bias term).

v2 changes vs baseline:
  - fp16 operands everywhere on-chip (PSUM accumulation stays fp32): halves DMA
    and SBUF traffic, 2x DVE throughput on RoPE elementwise ops, and allows
    128-wide matmuls at 1 cycle/row (fp32r would be 4x for moving<256).
  - fine-grained causal diagonal: crossing k-tiles only compute the valid q
    range (start=True resets the whole PSUM bank, zeroing the rest), and the
    triangular 128x128 sub-block mask is injected by one extra 128-moving
    matmul: psS += triA.T @ (-60000*I) adds -60000 exactly where q < k, so
    exp() flushes those to zero. This removes the gpsimd affine_select from
    the exp->PV critical path entirely and keeps masking PE-local.
  - V bias folded into the host-side combine (softmax weights sum to 1, so the
    bias contributes bv @ proj_w.T, a constant row); removes the normalize-time
    bias adds.
  - x is fully SBUF-resident; the qkv weights and the first x s-tile are
    host-packed into one DRAM tensor (8 fused startup DMAs), input DMAs are
    spread across the SP/Act HWDGE rings + gpsimd SWDGE queue by need-time,
    and the first s-tile's qkv runs dc-outer so it consumes chunks as they
    arrive.
  - proj(0)/proj(1)/proj(2) are partially deferred into the (filler-less)
    last attention's kt-loop, the last normalize window is covered by the
    reserved chunks, and the last q-tile's output DMAs fire per 512-column
    half across three queues as soon as each PSUM->SBUF copy lands.
  - proj PSUM shares a pool with the attention output accumulators (disjoint
    lifetimes), proj output is staged as one [128, 1024] tile per 128-row
    block -> one output DMA per block.
"""
import numpy as np

B, S, D = 2, 2048, 1024
HEADS, HD = 16, 64
HALF = HD // 2
NCORES = 8
GROUPS = 4          # tensor-parallel head groups per batch
HPG = HEADS // GROUPS
O_QK = 2 * HPG * HD   # 512 rows: [q h0..h3 | k h0..h3]
O_V = HPG * HD        # 256
SCALE = 1.0 / np.sqrt(HD)

NST = S // 512        # 4 seq tiles of 512
NDC = D // 128        # 8 contraction chunks
NKT = S // 128        # 16 k tiles

_NC_CACHE = None

# schedule knobs
PSS_BUFS = 2
E_BUFS = 7
RESERVE = 2   # fillers kept back to cover the normalize window
P1_BUFS = 3
NRM_BUFS = 2
SPLIT_EXP_QT3 = False
SPLIT_EXP_CROSS = False


def _build_nc():
    import concourse.bacc as bacc
    import concourse.mybir as mybir
    import concourse.tile as tile

    fp32 = mybir.dt.float32
    fp16 = mybir.dt.float16
    Exp = mybir.ActivationFunctionType.Exp

    nc = bacc.Bacc(trn_type="TRN2", target_bir_lowering=False, debug=False)

    xT = nc.dram_tensor("xT", [D, S], fp16, kind="ExternalInput").ap()
    wx0 = nc.dram_tensor("wx0", [D, O_QK + 512], fp16, kind="ExternalInput").ap()
    wvT = nc.dram_tensor("wvT", [D, O_V], fp16, kind="ExternalInput").ap()
    bqk = nc.dram_tensor("bqk", [128, 4], fp32, kind="ExternalInput").ap()
    cosF = nc.dram_tensor("cosF", [128, S], fp16, kind="ExternalInput").ap()
    sinF = nc.dram_tensor("sinF", [128, S], fp16, kind="ExternalInput").ap()
    pwT = nc.dram_tensor("pwT", [O_V, D], fp16, kind="ExternalInput").ap()
    triA = nc.dram_tensor("triA", [128, 128], fp16, kind="ExternalInput").ap()
    negI = nc.dram_tensor("negI", [128, 128], fp16, kind="ExternalInput").ap()
    outP = nc.dram_tensor("out_partial", [S, D], fp16, kind="ExternalOutput").ap()

    with tile.TileContext(nc) as tc:
        with tc.tile_pool(name="persist", bufs=1) as persist, \
             tc.tile_pool(name="p1sb", bufs=P1_BUFS) as p1sb, \
             tc.tile_pool(name="p2sb", bufs=E_BUFS) as p2sb, \
             tc.tile_pool(name="nrm", bufs=4) as nrm, \
             tc.tile_pool(name="p3sb", bufs=4) as p3sb, \
             tc.tile_pool(name="ps1", bufs=2, space="PSUM") as ps1_pool, \
             tc.tile_pool(name="psS", bufs=PSS_BUFS, space="PSUM") as psS_pool, \
             tc.tile_pool(name="psX", bufs=2, space="PSUM") as psX_pool:

            # --- persistent tiles -------------------------------------------
            # packed [wqk | x s-tile 0] per dc chunk: one startup DMA each
            wx0_t = [persist.tile([128, O_QK + 512], fp16, name=f"wx0{dc}", tag=f"wx0{dc}")
                     for dc in range(NDC)]
            wv_t = [persist.tile([128, O_V], fp16, name=f"wv{dc}", tag=f"wv{dc}")
                    for dc in range(NDC)]
            pw_t = [persist.tile([128, D], fp16, name=f"pw{dc}", tag=f"pw{dc}")
                    for dc in range(2)]
            xt_t = [persist.tile([128, S], fp16, name=f"xt{dc}", tag=f"xt{dc}")
                    for dc in range(NDC)]
            cos_t = persist.tile([128, S], fp16, name="cos", tag="cos")
            sin_t = persist.tile([128, S], fp16, name="sin", tag="sin")
            bqk_t = persist.tile([128, 4], fp32, name="bqk", tag="bqk")
            triA_t = persist.tile([128, 128], fp16, name="triA", tag="triA")
            negI_t = persist.tile([128, 128], fp16, name="negI", tag="negI")
            scratch = persist.tile([1, 2], fp32, name="scratch", tag="scratch")
            qk_t = [persist.tile([128, S], fp16, name=f"qk{ot}", tag=f"qk{ot}")
                    for ot in range(4)]
            vaug_t = [persist.tile([128, HPG * (HD + 1)], fp16, name=f"va{kt}", tag=f"va{kt}")
                      for kt in range(NKT)]
            outT_t = [persist.tile([128, S], fp16, name=f"oT{t}", tag=f"oT{t}")
                      for t in range(2)]

            # warm the Exp activation table while DMAs stream
            nc.gpsimd.memset(scratch[:], 0.0)
            nc.scalar.activation(scratch[:], scratch[:], Exp)

            # --- loads: spread across the SP / Act / DVE HWDGE rings so the
            # per-DMA fixed issue cost (~625ns) parallelizes; order by the
            # time each tensor is first needed.
            def dma_x(eng, st, dc):
                eng.dma_start(
                    out=xt_t[dc][:, st * 512:(st + 1) * 512],
                    in_=xT[dc * 128:(dc + 1) * 128, st * 512:(st + 1) * 512])

            # small early tensors on the gpsimd SWDGE queue (ready ~4us)
            nc.gpsimd.dma_start(out=bqk_t[:], in_=bqk)
            nc.gpsimd.dma_start(out=sin_t[:, 0:512], in_=sinF[:, 0:512])
            nc.gpsimd.dma_start(out=cos_t[:, 0:512], in_=cosF[:, 0:512])
            nc.gpsimd.dma_start(out=triA_t[:], in_=triA)
            nc.gpsimd.dma_start(out=negI_t[:], in_=negI)
            # packed qk-weights + first-s-tile x: big DMAs over both rings;
            # dc=0 is split in half across the rings so the first matmul's
            # inputs land as early as possible
            nc.sync.dma_start(out=wx0_t[0][:, 0:512], in_=wx0[0:128, 0:512])
            nc.scalar.dma_start(out=wx0_t[0][:, 512:1024], in_=wx0[0:128, 512:1024])
            for dc in range(1, NDC):
                eng = nc.sync if dc % 2 == 0 else nc.scalar
                eng.dma_start(out=wx0_t[dc][:],
                              in_=wx0[dc * 128:(dc + 1) * 128, :])
            # everything below streams during compute: keep it OFF the Act
            # ring (the Act sequencer must stay free for the softmax exps)
            for dc in range(NDC):
                eng = nc.gpsimd if dc % 2 == 0 else nc.sync
                eng.dma_start(out=wv_t[dc][:],
                              in_=wvT[dc * 128:(dc + 1) * 128, :])
            for dc in range(NDC):
                dma_x(nc.sync, 1, dc)
            nc.gpsimd.dma_start(out=pw_t[0][:], in_=pwT[0:128, :])
            nc.gpsimd.dma_start(out=pw_t[1][:], in_=pwT[128:256, :])
            nc.sync.dma_start(out=sin_t[:, 512:S], in_=sinF[:, 512:S])
            nc.sync.dma_start(out=cos_t[:, 512:S], in_=cosF[:, 512:S])
            for dc in range(NDC):
                nc.sync.dma_start(out=xt_t[dc][:, 1024:2048],
                                  in_=xT[dc * 128:(dc + 1) * 128, 1024:2048])
            for kt in range(NKT):
                nc.gpsimd.memset(vaug_t[kt][:], 1.0)

            def wqk_ap(dc, c0, c1):
                return wx0_t[dc][:, c0:c1]

            def x_ap(dc, e0, e1):
                # x element range [e0:e1) of the full row; s-tile 0 is packed
                if e1 <= 512:
                    return wx0_t[dc][:, O_QK + e0:O_QK + e1]
                return xt_t[dc][:, e0:e1]

            # --- phase-1 work, chunked for interleave -----------------------
            def rope_from(ps_ap, st, ot):
                """RoPE tail: bias + rotate ps_ap -> qk_t[ot][:, st*512:...]."""
                s0 = st * 512
                tb = p1sb.tile([128, 512], fp16, name=f"tb{st}_{ot}", tag="tb")
                nc.vector.tensor_scalar_add(tb[:], ps_ap, bqk_t[:, ot:ot + 1])
                m2s = p1sb.tile([128, 512], fp16, name=f"m2s{st}_{ot}", tag="m2s")
                nc.vector.tensor_mul(m2s[:], tb[:], sin_t[:, s0:s0 + 512])
                m2 = p1sb.tile([128, 512], fp16, name=f"m2{st}_{ot}", tag="m2")
                for band in range(4):
                    dstp = band * 32
                    srcp = (band ^ 1) * 32
                    nc.vector.tensor_copy(m2[dstp:dstp + 32, :], m2s[srcp:srcp + 32, :])
                m1 = p1sb.tile([128, 512], fp16, name=f"m1{st}_{ot}", tag="m2s")
                nc.vector.tensor_mul(m1[:], tb[:], cos_t[:, s0:s0 + 512])
                nc.vector.tensor_add(qk_t[ot][:, s0:s0 + 512], m1[:], m2[:])

            def emit_qk(st, ot):
                """One (o 128, s 512) q^T/k^T tile: matmul + bias + RoPE."""
                s0 = st * 512
                ps = ps1_pool.tile([128, 512], fp32, name=f"psA{st}_{ot}", tag="ps1")
                for dc in range(NDC):
                    nc.tensor.matmul(ps[:], wqk_ap(dc, ot * 128, (ot + 1) * 128),
                                     x_ap(dc, s0, s0 + 512),
                                     start=(dc == 0), stop=(dc == NDC - 1))
                rope_from(ps[:], st, ot)

            def phase1_start():
                """dc-outer qkv for s-tile 0: consume x chunks as they arrive.
                Uses the (idle at startup) psS pool as 4 parallel accumulators.
                psW[0] holds hp0's pair (ot0|ot2) so its slot frees early."""
                psW = [psS_pool.tile([128, 1024], fp32, name=f"psW{i}", tag="psS")
                       for i in range(2)]
                slot = {0: (0, 0), 2: (0, 512), 1: (1, 0), 3: (1, 512)}
                for dc in range(NDC):
                    for ot in (0, 2):
                        w, c0 = slot[ot]
                        nc.tensor.matmul(
                            psW[w][:, c0:c0 + 512],
                            wqk_ap(dc, ot * 128, (ot + 1) * 128),
                            x_ap(dc, 0, 512),
                            start=(dc == 0), stop=(dc == NDC - 1))
                # hp0's pair is complete: rope it while ot1/ot3 accumulate
                for ot in (0, 2):
                    w, c0 = slot[ot]
                    rope_from(psW[w][:, c0:c0 + 512], 0, ot)
                for dc in range(NDC):
                    for ot in (1, 3):
                        w, c0 = slot[ot]
                        nc.tensor.matmul(
                            psW[w][:, c0:c0 + 512],
                            wqk_ap(dc, ot * 128, (ot + 1) * 128),
                            x_ap(dc, 0, 512),
                            start=(dc == 0), stop=(dc == NDC - 1))
                for ss in range(4):
                    emit_v(0, ss)
                for ot in (1, 3):
                    w, c0 = slot[ot]
                    rope_from(psW[w][:, c0:c0 + 512], 0, ot)

            def emit_v(st, ss):
                """One (s 128, o 256) natural-layout V tile into vaug."""
                ps = ps1_pool.tile([128, O_V], fp32, name=f"psB{st}_{ss}", tag="ps1")
                s128 = st * 4 + ss
                for dc in range(NDC):
                    nc.tensor.matmul(ps[:], x_ap(dc, s128 * 128, (s128 + 1) * 128),
                                     wv_t[dc][:], start=(dc == 0), stop=(dc == NDC - 1))
                vt = vaug_t[s128]
                nc.vector.tensor_copy(
                    vt.rearrange("p (h w) -> p h w", w=HD + 1)[:, :, 0:HD],
                    ps.rearrange("p (h w) -> p h w", w=HD))

            def phase1_chunks(st):
                ch = []
                for ot in (0, 2, 1, 3):
                    ch.append(lambda ot=ot: emit_qk(st, ot))
                for ss in range(4):
                    ch.append(lambda ss=ss: emit_v(st, ss))
                return ch

            # --- attention for one q-tile, software-pipelined ---------------
            def attention(qt, filler, reserve=RESERVE):
                """filler: list of phase-1 chunk closures to interleave."""
                q0 = qt * 512
                kmax = (q0 + 512) // 128
                n_steps = 2 * kmax
                usable = max(0, len(filler) - reserve)
                fill_every = max(1, n_steps // usable) if usable else 0
                step = 0
                fired = 0

                def maybe_fill():
                    nonlocal step, fired
                    step += 1
                    if (filler and fill_every and fired < usable
                            and step % fill_every == 0):
                        fired += 1
                        filler.pop(0)()

                if qt == NST - 1:
                    # alternate long (non-crossing) and short (crossing) kts so
                    # the exp round-trip latency hides behind the long matmuls
                    cross = list(range(kmax - 4, kmax))
                    noncross = list(range(kmax - 4))
                    kseq = [noncross[0]]
                    rest = noncross[1:]
                    for i, c in enumerate(cross):
                        if i < len(rest):
                            kseq.append(rest[i])
                        kseq.append(c)
                    kseq += rest[len(cross):]
                else:
                    kseq = list(range(kmax))
                for hp in range(2):
                    q_tile = qk_t[hp]
                    k_tile = qk_t[2 + hp]
                    hA, hB = 2 * hp, 2 * hp + 1
                    psO_A = psX_pool.tile([128, 512], fp32, name=f"psOA{hp}_{qt}", tag="psX")
                    psO_B = psX_pool.tile([128, 512], fp32, name=f"psOB{hp}_{qt}", tag="psX")
                    e_tiles = [None] * kmax
                    assert kseq[0] == 0 or qt == 0  # first PV must cover all cols

                    def emit_st(kt):
                        k0 = kt * 128
                        jj = (k0 - q0) // 128  # >= 0 means diagonal-crossing
                        psS = psS_pool.tile([128, 1024], fp32, name=f"psS{hp}_{qt}_{kt}", tag="psS")
                        e = p2sb.tile([128, 1024], fp16, name=f"e{hp}_{qt}_{kt}", tag="e")
                        if jj < 0:
                            nc.tensor.matmul(psS[:, 0:512], k_tile[0:64, k0:k0 + 128],
                                             q_tile[0:64, q0:q0 + 512], start=True, stop=True,
                                             tile_position=(0, 0))
                            nc.tensor.matmul(psS[:, 512:1024], k_tile[64:128, k0:k0 + 128],
                                             q_tile[64:128, q0:q0 + 512], start=True, stop=True,
                                             tile_position=(64, 0))
                            if qt == NST - 1 and SPLIT_EXP_QT3:
                                # no fillers in the last q-tile: split the exp
                                # per head so each PV waits on half the latency
                                nc.scalar.activation(e[:, 0:512], psS[:, 0:512],
                                                     Exp, scale=float(SCALE))
                                nc.scalar.activation(e[:, 512:1024], psS[:, 512:1024],
                                                     Exp, scale=float(SCALE))
                            else:
                                nc.scalar.activation(e[:], psS[:], Exp, scale=float(SCALE))
                        else:
                            lo = jj * 128
                            # valid q range in one matmul per head; start=True
                            # resets the whole bank (zeroing the unwritten part).
                            # Head A fully (scores+mask) before head B so exp_A
                            # can start while B's scores run.
                            nc.tensor.matmul(psS[:, lo:512],
                                             k_tile[0:64, k0:k0 + 128],
                                             q_tile[0:64, q0 + lo:q0 + 512],
                                             start=True, stop=False, tile_position=(0, 0),
                                             skip_group_check=True)
                            nc.tensor.matmul(psS[:, lo:lo + 128], triA_t[:], negI_t[:],
                                             start=False, stop=True,
                                             skip_group_check=True)
                            nc.tensor.matmul(psS[:, 512 + lo:1024],
                                             k_tile[64:128, k0:k0 + 128],
                                             q_tile[64:128, q0 + lo:q0 + 512],
                                             start=True, stop=False, tile_position=(64, 0),
                                             skip_group_check=True)
                            nc.tensor.matmul(psS[:, 512 + lo:512 + lo + 128],
                                             triA_t[:], negI_t[:],
                                             start=False, stop=True,
                                             skip_group_check=True)
                            if SPLIT_EXP_CROSS:
                                nc.scalar.activation(e[:, lo:512], psS[:, lo:512],
                                                     Exp, scale=float(SCALE))
                                nc.scalar.activation(e[:, 512 + lo:1024],
                                                     psS[:, 512 + lo:1024],
                                                     Exp, scale=float(SCALE))
                            else:
                                pv = psS.rearrange("p (h q) -> p h q", q=512)[:, :, lo:512]
                                ev = e.rearrange("p (h q) -> p h q", q=512)[:, :, lo:512]
                                nc.scalar.activation(ev, pv, Exp, scale=float(SCALE))
                        e_tiles[kt] = e

                    def emit_pv(kt, st_f, sp_f):
                        k0 = kt * 128
                        jj = (k0 - q0) // 128
                        lo = jj * 128 if jj > 0 else 0
                        e = e_tiles[kt]
                        nc.tensor.matmul(psO_A[0:HD + 1, lo:512],
                                         vaug_t[kt][:, hA * 65:hA * 65 + 65],
                                         e[:, lo:512], start=st_f, stop=sp_f,
                                         skip_group_check=True)
                        nc.tensor.matmul(psO_B[0:HD + 1, lo:512],
                                         vaug_t[kt][:, hB * 65:hB * 65 + 65],
                                         e[:, 512 + lo:1024], start=st_f, stop=sp_f,
                                         skip_group_check=True)

                    emit_st(kseq[0])
                    for i, kt in enumerate(kseq):
                        if i + 1 < kmax:
                            emit_st(kseq[i + 1])
                        emit_pv(kt, st_f=(i == 0), sp_f=(i == kmax - 1))
                        maybe_fill()

                    for idx, psO in ((0, psO_A), (1, psO_B)):
                        rec = nrm.tile([1, 512], fp32, name=f"rec{hp}_{qt}_{idx}", tag="rec")
                        nc.vector.reciprocal(rec[0:1, :], psO[HD:HD + 1, :])
                        rbc = nrm.tile([64, 512], fp32, name=f"rbc{hp}_{qt}_{idx}", tag="rbc")
                        nc.gpsimd.partition_broadcast(rbc[:], rec[0:1, :])
                        nc.vector.tensor_mul(
                            outT_t[hp][idx * 64:(idx + 1) * 64, q0:q0 + 512],
                            psO[0:64, :], rbc[:])
                while filler:
                    filler.pop(0)()

            # --- output projection for one q-tile ---------------------------
            def proj(qt, ssts=(0, 1, 2, 3), pool=None):
                for sst in ssts:
                    st128 = qt * 4 + sst
                    po = p3sb.tile([128, 1024], fp16, name=f"po{st128}", tag="po")
                    pp = pool or psX_pool
                    for jt in range(2):
                        ps = pp.tile([128, 512], fp32, name=f"psP{st128}_{jt}",
                                     tag="ps1" if pool else "psX")
                        for dc in range(2):
                            nc.tensor.matmul(ps[:], outT_t[dc][:, st128 * 128:(st128 + 1) * 128],
                                             pw_t[dc][:, jt * 512:(jt + 1) * 512],
                                             start=(dc == 0), stop=(dc == 1))
                        if jt == 0:
                            nc.vector.tensor_copy(po[:, 0:512], ps[:])
                        elif pool is not None:
                            nc.vector.tensor_copy(po[:, 512:1024], ps[:])
                        else:
                            nc.scalar.copy(po[:, 512:1024], ps[:])
                        if qt == NST - 1:
                            # fire each half's DMA as soon as its copy lands
                            eng = ((nc.sync, nc.gpsimd, nc.sync, nc.gpsimd),
                                   (nc.gpsimd, nc.scalar, nc.scalar, nc.sync))[jt][sst]
                            eng.dma_start(
                                out=outP[st128 * 128:(st128 + 1) * 128,
                                         jt * 512:(jt + 1) * 512],
                                in_=po[:, jt * 512:(jt + 1) * 512])
                    # spread output DMAs across all three queues so the final
                    # drain parallelizes (inputs are done by the time these go)
                    if qt != NST - 1:
                        eng = (nc.gpsimd, nc.sync, nc.gpsimd, nc.sync)[sst]
                        eng.dma_start(
                            out=outP[st128 * 128:(st128 + 1) * 128, :], in_=po[:])

            # --- main schedule ----------------------------------------------
            phase1_start()
            for qt in range(NST):
                if qt + 1 < NST:
                    filler = phase1_chunks(qt + 1)
                elif qt == NST - 1:
                    # the last attention tile is Act(exp)-paced and has no
                    # phase-1 left: feed it ALL deferred projection blocks
                    # (ps1 pool is idle by then) to balance PE against Act
                    filler = [lambda q=q, s=s: proj(q, ssts=(s,), pool=ps1_pool)
                              for q in (2, 1, 0) for s in range(4)]
                attention(qt, filler, reserve=6 if qt == NST - 1 else RESERVE)
                if qt == NST - 1:
                    proj(qt)

    nc.compile()
    return nc


def _get_nc():
    global _NC_CACHE
    if _NC_CACHE is None:
        _NC_CACHE = _build_nc()
    return _NC_CACHE


def _prep_in_maps(x, qkv_w, qkv_b, proj_w, proj_b):
    x = np.asarray(x, dtype=np.float32)
    qkv_w = np.asarray(qkv_w, dtype=np.float32)
    qkv_b = np.asarray(qkv_b, dtype=np.float32)
    proj_w = np.asarray(proj_w, dtype=np.float32)

    # RoPE tables (match reference fp32 math, then cast to fp16)
    freqs = (1.0 / (10000.0 ** (np.arange(HALF, dtype=np.float32) / HALF))).astype(np.float32)
    pos = np.arange(S, dtype=np.float32)
    ang = pos[:, None] * freqs[None, :]            # (S, 32)
    cos_m = np.cos(ang).astype(np.float32)
    sin_m = np.sin(ang).astype(np.float32)
    p = np.arange(128)
    cosF = np.ascontiguousarray(cos_m[:, p % HALF].T).astype(np.float16)   # (128, S)
    sign = np.where((p % HD) < HALF, 1.0, -1.0).astype(np.float32)
    sinF = np.ascontiguousarray((sin_m[:, p % HALF] * sign[None, :]).T).astype(np.float16)

    # causal mask via PE: (triA.T @ negI)[p, f] = -60000 * (f < p)
    kk, pp2 = np.meshgrid(np.arange(128), np.arange(128), indexing="ij")
    triA = (pp2 > kk).astype(np.float16)             # [k, p] = 1 where p > k
    negI = (-60000.0 * np.eye(128)).astype(np.float16)

    in_maps = []
    for c in range(NCORES):
        b, g = divmod(c, GROUPS)
        hs = np.arange(HPG) + HPG * g
        q_rows = np.concatenate([np.arange(HD * h, HD * h + HD) for h in hs])
        qk_rows = np.concatenate([q_rows, q_rows + D])
        in_maps.append({
            "xT": np.ascontiguousarray(x[b].T).astype(np.float16),
            "wx0": np.ascontiguousarray(
                np.concatenate([qkv_w[qk_rows, :].T, x[b].T[:, 0:512]], axis=1)
            ).astype(np.float16),
            "wvT": np.ascontiguousarray(qkv_w[q_rows + 2 * D, :].T).astype(np.float16),
            "bqk": np.ascontiguousarray(qkv_b[qk_rows].reshape(4, 128).T),
            "cosF": cosF,
            "sinF": sinF,
            "pwT": np.ascontiguousarray(proj_w[:, q_rows].T).astype(np.float16),
            "triA": triA,
            "negI": negI,
        })
    return in_maps


def _combine(results, qkv_b, proj_w, proj_b):
    """Sum per-core partials; add proj bias and the folded V-bias term."""
    qkv_b = np.asarray(qkv_b, dtype=np.float32)
    proj_w = np.asarray(proj_w, dtype=np.float32)
    proj_b = np.asarray(proj_b, dtype=np.float32)
    bv_fold = proj_w @ qkv_b[2 * D:3 * D]          # (D,)
    const = (proj_b + bv_fold).astype(np.float64)
    out = np.empty((B, S, D), dtype=np.float32)
    for b in range(B):
        acc = np.zeros((S, D), dtype=np.float64)
        for g in range(GROUPS):
            acc += results[b * GROUPS + g]["out_partial"].astype(np.float64)
        out[b] = (acc + const[None, :]).astype(np.float32)
    return out


def _run(x, qkv_w, qkv_b, proj_w, proj_b, trace=False):
    from concourse import bass_utils
    nc = _get_nc()
    in_maps = _prep_in_maps(x, qkv_w, qkv_b, proj_w, proj_b)
    res = bass_utils.run_bass_kernel_spmd(nc, in_maps, core_ids=list(range(NCORES)),
                                          trace=trace)
    out = _combine(res.results, qkv_b, proj_w, proj_b)
    return out, res


def kernel(x, qkv_w, qkv_b, proj_w, proj_b):
    out, _ = _run(x, qkv_w, qkv_b, proj_w, proj_b, trace=False)
    return out
